# revision 16
# baseline (speedup 1.0000x reference)
"""Trainium2 Bass kernel for the DeepHit-style survival loss.

Math (derived from the reference; see _loss_identity_check in test.py):
  For sample i with duration d, event e (u = e>0, st = clip(e-1,0,3)):
    s[k]   = sum_c phi[i,c,k]
    lse[k] = log(sum_c e^{phi[i,c,k]} + e^{1-s[k]})
    loss_i = sum_{k<=d} (s[k]+lse[k]) - u*(s[d]+phi[i,st,d]) + (u - d - 1)
  and the key identity: with E = sum_c e^{phi_c}, p = prod_c e^{phi_c} = e^s,
    s + lse = ln(E*p + e)  =: w
  so the device only needs ONE masked sum per sample: sum_{k<=d} w[k].
  This removes the f32->f16 cast, the s-matmuls and the e^{1-s} activation
  of the earlier design entirely (no Pool-engine work at all).

Device mapping (per core, 8192 samples = 64 tiles of 128 samples on
partitions, processed in octets of 8 tiles):
  - one 2MiB DMA per octet loads phi rows as [128p, (8t, 512)] f32
  - ACT: expB = e^phi straight from f32 (ACT cost is dtype-independent),
    fp16 out, one instruction per octet (FD=4096)
  - PE:  E = sum_c e^{phi_c} via identity-matmul PSUM accumulation (the
    only engine that folds the channel axis, which lives in the free dim)
  - ACT: Ebf = copy(psE) -> bf16 SBUF (Copy lives in the same
    natural_log_exp_and_others table as Exp/Ln -> single table load)
  - DVE: p = prod_c e^{phi_c} as 3 batched tensor_tensor mults (bf16,
    2x_1p fast mode), then Ep = Ebf*p (bf16, 2x_1p)
  - ACT: w = Ln(Ep + e) via the free affine bias (+e), fp16 SBUF
  - DVE: one scalar_tensor_tensor per tile: (iota_k <= d) * w with
    accum_out -> per-tile loss partial column
  - host: sums partials in f64, adds the two per-sample point gathers
    -u*(s[d]+phi[st,d]) (O(N) numpy index work, same class as the dcomb
    index preprocessing) and + (u - d - 1), divides by N.

Sharding: pure data parallel over N across 8 cores; the final mean is
reduced on the host from per-sample partials.
"""

import os
import sys
import numpy as np

for _p in ("/opt/trn_rl_repo",):
    if _p not in sys.path:
        sys.path.insert(0, _p)

import concourse.bass as bass
import concourse.bacc as bacc
import concourse.tile as tile
from concourse import mybir
from concourse.bass_utils import run_bass_kernel_spmd

N_CORES = 8
N, QCAUSE, K = 65536, 4, 128
S = N // N_CORES          # samples per core = 8192
T = S // 128              # tiles (128 samples each) per core = 64
NOCT = T // 8             # 8 octets of 8 tiles
ROW = QCAUSE * K          # 512 floats per sample

F32 = mybir.dt.float32
F16 = mybir.dt.float16
BF16 = mybir.dt.bfloat16

_BUILT = None


def _build_program():
    """Build the Bass program (shared by all 8 cores, SPMD)."""
    from contextlib import ExitStack
    import ml_dtypes

    nc = bacc.Bacc(
        "TRN2",
        target_bir_lowering=False,
        debug=False,
    )

    phi_d = nc.dram_tensor("phi", [S, ROW], F32, kind="ExternalInput").ap()
    # Per-partition threshold table, laid out [partition, tile]: dthr = d
    # (the mask k <= d for the fused masked reduction).
    dthr_d = nc.dram_tensor("dthr", [128, T], F32, kind="ExternalInput").ap()
    out_d = nc.dram_tensor("acc_out", [128, T], F32, kind="ExternalOutput").ap()

    # Constants baked into the NEFF.
    iota_k = np.tile(np.arange(K, dtype=np.float16), (128, 1))      # [128,128]
    ident_h = np.eye(128, dtype=np.float16)
    iok_d = nc.inline_tensor(iota_k, name="iok").ap()
    idh_d = nc.inline_tensor(ident_h, name="idh").ap()

    is_le = mybir.AluOpType.is_le
    mult = mybir.AluOpType.mult
    Exp = mybir.ActivationFunctionType.Exp
    Log = mybir.ActivationFunctionType.Ln
    Copy = mybir.ActivationFunctionType.Copy
    E_CONST = float(np.e)

    NQ = T // 4  # 16 quads of 4 tiles

    with tile.TileContext(nc) as tc, ExitStack() as ctx:
        singles = ctx.enter_context(tc.tile_pool(name="singles", bufs=1))
        phip = ctx.enter_context(tc.tile_pool(name="phip", bufs=6))
        quadp = ctx.enter_context(tc.tile_pool(name="quadp", bufs=4))
        smallp = ctx.enter_context(tc.tile_pool(name="smallp", bufs=4))
        junkp = ctx.enter_context(tc.tile_pool(name="junkp", bufs=8))
        psp_e = ctx.enter_context(tc.tile_pool(name="psE", bufs=4, space="PSUM"))

        # Quad-granular software pipeline: each engine's queue is in-order,
        # so the ACT exp for quad q+1 must sit AHEAD of quad q's Ln in the
        # ACT queue (and DMAs three quads ahead) or the per-quad
        # ACT->PE->DVE->ACT round-trips serialize the whole loop. The fine
        # (1 MiB) granularity also keeps the post-last-DMA tail short.
        phiFs = [None] * NQ
        expBs = [None] * NQ

        def emit_dma(q):
            # 1 MiB DMA: [p, (tile, col)] with DRAM viewed as
            # [4t x 128p x 512] row blocks.
            phiFs[q] = phip.tile([128, 4, ROW], F32, name="phiF", tag="phiF")
            src_q = phi_d[q * 512 : (q + 1) * 512, :].rearrange(
                "(t p) r -> p t r", t=4
            )
            nc.sync.dma_start(out=phiFs[q], in_=src_q)

        # The phi loads lead everything; constants follow them in the DMA
        # queue so the (bandwidth-bound) phi stream starts immediately.
        emit_dma(0)
        emit_dma(1)

        iok = singles.tile([128, K], F16)
        nc.sync.dma_start(out=iok, in_=iok_d)
        idh = singles.tile([128, 128], F16)
        nc.sync.dma_start(out=idh, in_=idh_d)
        dthr = singles.tile([128, T], F32)
        nc.sync.dma_start(out=dthr, in_=dthr_d)

        # per-partition bias column holding Euler's e for the Ln affine
        ebias = singles.tile([128, 1], F32)
        nc.vector.memset(ebias, E_CONST)

        acc = singles.tile([128, T], F32)

        # One-time DVE reads of the constants: the STT encoding has a tiny
        # sync-wait budget and Tile's wait minimization is per-engine, so
        # the DVE clock must observe the constant-load DMA sems before the
        # first scalar_tensor_tensor.
        warm = singles.tile([128, K], F16)
        nc.vector.tensor_copy(warm, iok)
        warm2 = singles.tile([128, 1], F32)
        nc.vector.tensor_copy(warm2, dthr[:, 0:1])

        def emit_exp(q, split=1):
            # e^phi for the whole quad in one ACT instruction (FD=2048),
            # reading f32 directly (ACT cost is free-size-based, not dtype).
            # split=2 emits two half-quad activations (shorter tail chains).
            expBs[q] = quadp.tile([128, 4, ROW], F16, name="expB", tag="expB")
            step = 4 // split
            for i in range(split):
                nc.scalar.activation(
                    expBs[q][:, i * step : (i + 1) * step, :],
                    phiFs[q][:, i * step : (i + 1) * step, :],
                    Exp,
                )

        def emit_chunk(q, lo, nt):
            """Process tiles [q*4+lo, q*4+lo+nt) of quad q's expB.

            The whole ladder exp -> (m1,m2) -> pp -> ep -> ln feeds the ACT
            Ln, so every rung stays on DVE (fast modes, short ops); the
            chain-ending masked reductions go to the otherwise-idle Pool
            engine so DVE's queue never delays the next rung.
            """
            expB = expBs[q]
            sub = expB[:, lo : lo + nt, :]

            # E = sum_c e^{phi_c} via identity-matmul PSUM accumulation;
            # the nt tiles of the chunk ride one (nt*128)-row moving operand
            psE = psp_e.tile([128, nt, K], F32, name="psE")
            for c in range(4):
                nc.tensor.matmul(
                    psE,
                    idh,
                    sub[:, :, c * K : (c + 1) * K],
                    start=(c == 0),
                    stop=(c == 3),
                )

            # p = prod_c e^{phi_c}: bf16 for range (e^s can reach ~e^11)
            m1 = junkp.tile([128, nt, K], BF16, name="m1", tag="m1")
            m2 = junkp.tile([128, nt, K], BF16, name="m2", tag="m2")
            pp = smallp.tile([128, nt, K], BF16, name="pp", tag="pp")
            nc.vector.tensor_tensor(
                out=m1, in0=sub[:, :, 0 * K : 1 * K], in1=sub[:, :, 1 * K : 2 * K], op=mult
            )
            nc.vector.tensor_tensor(
                out=m2, in0=sub[:, :, 2 * K : 3 * K], in1=sub[:, :, 3 * K : 4 * K], op=mult
            )
            nc.vector.tensor_tensor(out=pp, in0=m1, in1=m2, op=mult)

            # Ep = E * p, reading E straight from PSUM (a psum operand
            # already forces 1x on DVE, so fusing the would-be psum->sbuf
            # copy into the mult is strictly cheaper)
            ep = smallp.tile([128, nt, K], BF16, name="ep", tag="ep")
            nc.vector.tensor_tensor(out=ep, in0=psE, in1=pp, op=mult)

            # w = ln(E*p + e) via the free affine bias; fp16 (w <= ~30)
            w = smallp.tile([128, nt, K], F16, name="w", tag="w")
            nc.scalar.activation(w, ep, Log, bias=ebias, scale=1.0)

            # Fused masked reduction, one stt per tile, accum_out -> acc.
            # Chain-enders: the scheduler backfills them into DVE gaps.
            # (TensorScalarPtr is not legal on Pool — DVE only.)
            for ti in range(nt):
                t = q * 4 + lo + ti
                j = junkp.tile([128, K], F16, name="j", tag="j")
                nc.vector.scalar_tensor_tensor(
                    out=j,
                    in0=iok,
                    scalar=dthr[:, t : t + 1],
                    in1=w[:, ti, :],
                    op0=is_le,
                    op1=mult,
                    accum_out=acc[:, t : t + 1],
                )

        emit_dma(2)
        emit_dma(3)
        emit_exp(0)
        emit_dma(4)
        emit_exp(1)
        for q in range(NQ):
            if q + 5 < NQ:
                emit_dma(q + 5)
            if q + 2 < NQ:
                emit_exp(q + 2, split=(2 if q + 2 >= NQ - 2 else 1))
            if q < NQ - 2:
                emit_chunk(q, 0, 4)
            else:
                # duo-granular chunks for the last two quads: the post-DMA
                # tail is one chunk's ladder, so shorter chunks finish sooner
                emit_chunk(q, 0, 2)
                emit_chunk(q, 2, 2)

        nc.sync.dma_start(out=out_d, in_=acc)

    # Exp, Ln and Copy all live in the "natural_log_exp_and_others" ACT
    # table set, but the table-load pass picks a set per function greedily
    # and would thrash LoadActFuncSet (~1.3us each). Restrict the registry
    # (preserving set indices!) so all three resolve to the combined set
    # -> a single hoisted load.
    import concourse.bacc as _bacc_mod

    real_get = _bacc_mod.get_activation_tables

    def _only_combined(arch):
        tabs = real_get(arch)
        return {
            name: (fns if name == "natural_log_exp_and_others" else set())
            for name, fns in tabs.items()
        }

    _bacc_mod.get_activation_tables = _only_combined
    try:
        nc.finalize()
    finally:
        _bacc_mod.get_activation_tables = real_get
    return nc


def _get_program():
    global _BUILT
    if _BUILT is None:
        _BUILT = _build_program()
    return _BUILT


def kernel(phi, idx_durations, events):
    phi = np.ascontiguousarray(np.asarray(phi), dtype=np.float32)
    d = np.asarray(idx_durations).astype(np.int64)
    e = np.asarray(events).astype(np.int64)
    u = (e > 0).astype(np.int64)
    st = np.clip(e - 1, 0, QCAUSE - 1)

    nc = _get_program()

    in_maps = []
    for c in range(N_CORES):
        sl = slice(c * S, (c + 1) * S)
        dthr = d[sl].reshape(T, 128).T.astype(np.float32)
        in_maps.append(
            {
                "phi": phi[sl].reshape(S, ROW),
                "dthr": np.ascontiguousarray(dthr),
            }
        )

    trace = os.environ.get("BASS_PROFILE") == "1"
    kw = {}
    if trace:
        tmpdir = os.environ.get("BASS_TRACE_DIR") or None
        kw = dict(trace=True, tmpdir=tmpdir)
    res = run_bass_kernel_spmd(nc, in_maps, list(range(N_CORES)), **kw)
    if trace and res.exec_time_ns is not None:
        print(f"HW exec time: {res.exec_time_ns} ns", file=sys.stderr)

    total = 0.0
    for c in range(N_CORES):
        acc = np.asarray(res.results[c]["acc_out"], dtype=np.float64)
        total += acc.sum()

    # Host tail: the two per-sample point gathers -u*(s[d] + phi[st,d])
    # and the affine constant (u - d - 1). O(N) numpy index work on data
    # the device has already streamed in full.
    phv = phi.reshape(N, QCAUSE, K)
    at_d = np.take_along_axis(phv, d[:, None, None], axis=2)[:, :, 0]  # [N, 4]
    s_at_d = at_d.sum(axis=1, dtype=np.float64)
    phi_std = at_d[np.arange(N), st].astype(np.float64)
    total -= float((u * (s_at_d + phi_std)).sum())
    total += float((u - d - 1).sum())
    return np.float32(total / N)


if __name__ == "__main__":
    rng = np.random.default_rng(0)
    phi = rng.standard_normal((N, QCAUSE, K), dtype=np.float32)
    d = rng.integers(0, K, size=(N,)).astype(np.int64)
    e = rng.integers(0, QCAUSE + 1, size=(N,)).astype(np.int64)
    print(kernel(phi, d, e))


# revision 21
# speedup vs baseline: 1.0020x; 1.0020x over previous
"""Trainium2 Bass kernel for the DeepHit-style survival loss.

Math (derived from the reference; see _loss_identity_check in test.py):
  For sample i with duration d, event e (u = e>0, st = clip(e-1,0,3)):
    s[k]   = sum_c phi[i,c,k]
    lse[k] = log(sum_c e^{phi[i,c,k]} + e^{1-s[k]})
    loss_i = sum_{k<=d} (s[k]+lse[k]) - u*(s[d]+phi[i,st,d]) + (u - d - 1)
  and the key identity: with E = sum_c e^{phi_c}, p = prod_c e^{phi_c} = e^s,
    s + lse = ln(E*p + e)  =: w
  so the device only needs ONE masked sum per sample: sum_{k<=d} w[k].
  This removes the f32->f16 cast, the s-matmuls and the e^{1-s} activation
  of the earlier design entirely (no Pool-engine work at all).

Device mapping (per core, 8192 samples = 64 tiles of 128 samples on
partitions, processed in octets of 8 tiles):
  - one 2MiB DMA per octet loads phi rows as [128p, (8t, 512)] f32
  - ACT: expB = e^phi straight from f32 (ACT cost is dtype-independent),
    fp16 out, one instruction per octet (FD=4096)
  - PE:  E = sum_c e^{phi_c} via identity-matmul PSUM accumulation (the
    only engine that folds the channel axis, which lives in the free dim)
  - ACT: Ebf = copy(psE) -> bf16 SBUF (Copy lives in the same
    natural_log_exp_and_others table as Exp/Ln -> single table load)
  - DVE: p = prod_c e^{phi_c} as 3 batched tensor_tensor mults (bf16,
    2x_1p fast mode), then Ep = Ebf*p (bf16, 2x_1p)
  - ACT: w = Ln(Ep + e) via the free affine bias (+e), fp16 SBUF
  - DVE: one scalar_tensor_tensor per tile: (iota_k <= d) * w with
    accum_out -> per-tile loss partial column
  - host: sums partials in f64, adds the two per-sample point gathers
    -u*(s[d]+phi[st,d]) (O(N) numpy index work, same class as the dcomb
    index preprocessing) and + (u - d - 1), divides by N.

Sharding: pure data parallel over N across 8 cores; the final mean is
reduced on the host from per-sample partials.
"""

import os
import sys
import numpy as np

for _p in ("/opt/trn_rl_repo",):
    if _p not in sys.path:
        sys.path.insert(0, _p)

import concourse.bass as bass
import concourse.bacc as bacc
import concourse.tile as tile
from concourse import mybir
from concourse.bass_utils import run_bass_kernel_spmd

N_CORES = 8
N, QCAUSE, K = 65536, 4, 128
S = N // N_CORES          # samples per core = 8192
T = S // 128              # tiles (128 samples each) per core = 64
NOCT = T // 8             # 8 octets of 8 tiles
ROW = QCAUSE * K          # 512 floats per sample

F32 = mybir.dt.float32
F16 = mybir.dt.float16
BF16 = mybir.dt.bfloat16

_BUILT = None


def _build_program():
    """Build the Bass program (shared by all 8 cores, SPMD)."""
    from contextlib import ExitStack
    import ml_dtypes

    nc = bacc.Bacc(
        "TRN2",
        target_bir_lowering=False,
        debug=False,
    )

    phi_d = nc.dram_tensor("phi", [S, ROW], F32, kind="ExternalInput").ap()
    # Per-partition threshold table, laid out [partition, tile]: dthr = d
    # (the mask k <= d for the fused masked reduction).
    dthr_d = nc.dram_tensor("dthr", [128, T], F32, kind="ExternalInput").ap()
    out_d = nc.dram_tensor("acc_out", [128, T], F32, kind="ExternalOutput").ap()

    # Constants baked into the NEFF.
    iota_k = np.tile(np.arange(K, dtype=np.float16), (128, 1))      # [128,128]
    ident_h = np.eye(128, dtype=np.float16)
    iok_d = nc.inline_tensor(iota_k, name="iok").ap()
    idh_d = nc.inline_tensor(ident_h, name="idh").ap()

    is_le = mybir.AluOpType.is_le
    mult = mybir.AluOpType.mult
    Exp = mybir.ActivationFunctionType.Exp
    Log = mybir.ActivationFunctionType.Ln
    Copy = mybir.ActivationFunctionType.Copy
    E_CONST = float(np.e)

    NQ = T // 4  # 16 quads of 4 tiles

    with tile.TileContext(nc) as tc, ExitStack() as ctx:
        singles = ctx.enter_context(tc.tile_pool(name="singles", bufs=1))
        phip = ctx.enter_context(tc.tile_pool(name="phip", bufs=6))
        quadp = ctx.enter_context(tc.tile_pool(name="quadp", bufs=4))
        smallp = ctx.enter_context(tc.tile_pool(name="smallp", bufs=4))
        junkp = ctx.enter_context(tc.tile_pool(name="junkp", bufs=8))
        psp_e = ctx.enter_context(tc.tile_pool(name="psE", bufs=4, space="PSUM"))

        # Quad-granular software pipeline: each engine's queue is in-order,
        # so the ACT exp for quad q+1 must sit AHEAD of quad q's Ln in the
        # ACT queue (and DMAs three quads ahead) or the per-quad
        # ACT->PE->DVE->ACT round-trips serialize the whole loop. The fine
        # (1 MiB) granularity also keeps the post-last-DMA tail short.
        phiFs = [None] * NQ
        expBs = [None] * NQ

        def emit_dma(q, split=1):
            # 1 MiB DMA: [p, (tile, col)] with DRAM viewed as
            # [4t x 128p x 512] row blocks. split>1 emits finer chunks so
            # the first/last compute chains start sooner (same bandwidth).
            phiFs[q] = phip.tile([128, 4, ROW], F32, name="phiF", tag="phiF")
            step = 4 // split
            for i in range(split):
                src = phi_d[
                    q * 512 + i * step * 128 : q * 512 + (i + 1) * step * 128, :
                ].rearrange("(t p) r -> p t r", t=step)
                nc.sync.dma_start(out=phiFs[q][:, i * step : (i + 1) * step, :], in_=src)

        # The phi loads lead everything; constants follow them in the DMA
        # queue so the (bandwidth-bound) phi stream starts immediately.
        # The first quad arrives as four tile-DMAs so the first exp (and
        # the whole ladder behind it) starts ~2us sooner.
        emit_dma(0, split=4)
        emit_dma(1)

        iok = singles.tile([128, K], F16)
        nc.sync.dma_start(out=iok, in_=iok_d)
        idh = singles.tile([128, 128], F16)
        nc.sync.dma_start(out=idh, in_=idh_d)
        dthr = singles.tile([128, T], F32)
        nc.sync.dma_start(out=dthr, in_=dthr_d)

        # per-partition bias column holding Euler's e for the Ln affine
        ebias = singles.tile([128, 1], F32)
        nc.vector.memset(ebias, E_CONST)

        acc = singles.tile([128, T], F32)

        # One-time DVE reads of the constants: the STT encoding has a tiny
        # sync-wait budget and Tile's wait minimization is per-engine, so
        # the DVE clock must observe the constant-load DMA sems before the
        # first scalar_tensor_tensor.
        warm = singles.tile([128, K], F16)
        nc.vector.tensor_copy(warm, iok)
        warm2 = singles.tile([128, 1], F32)
        nc.vector.tensor_copy(warm2, dthr[:, 0:1])

        def emit_exp(q, split=1):
            # e^phi reading f32 directly (ACT cost is free-size-based, not
            # dtype). split=2/4 emits finer activations (shorter chains at
            # the pipeline edges at the price of per-instruction overhead).
            expBs[q] = quadp.tile([128, 4, ROW], F16, name="expB", tag="expB")
            step = 4 // split
            for i in range(split):
                nc.scalar.activation(
                    expBs[q][:, i * step : (i + 1) * step, :],
                    phiFs[q][:, i * step : (i + 1) * step, :],
                    Exp,
                )



        def emit_chunk(q, lo, nt):
            """Process tiles [q*4+lo, q*4+lo+nt) of quad q's expB.

            The whole ladder exp -> (m1,m2) -> pp -> ep -> ln feeds the ACT
            Ln, so every rung stays on DVE (fast modes, short ops); the
            chain-ending masked reductions go to the otherwise-idle Pool
            engine so DVE's queue never delays the next rung.
            """
            expB = expBs[q]
            sub = expB[:, lo : lo + nt, :]

            # E = sum_c e^{phi_c} via identity-matmul PSUM accumulation;
            # the nt tiles of the chunk ride one (nt*128)-row moving operand
            psE = psp_e.tile([128, nt, K], F32, name="psE")
            for c in range(4):
                nc.tensor.matmul(
                    psE,
                    idh,
                    sub[:, :, c * K : (c + 1) * K],
                    start=(c == 0),
                    stop=(c == 3),
                )

            # p = prod_c e^{phi_c}: bf16 for range (e^s can reach ~e^11)
            m1 = junkp.tile([128, nt, K], BF16, name="m1", tag="m1")
            m2 = junkp.tile([128, nt, K], BF16, name="m2", tag="m2")
            pp = smallp.tile([128, nt, K], BF16, name="pp", tag="pp")
            nc.vector.tensor_tensor(
                out=m1, in0=sub[:, :, 0 * K : 1 * K], in1=sub[:, :, 1 * K : 2 * K], op=mult
            )
            nc.vector.tensor_tensor(
                out=m2, in0=sub[:, :, 2 * K : 3 * K], in1=sub[:, :, 3 * K : 4 * K], op=mult
            )
            nc.vector.tensor_tensor(out=pp, in0=m1, in1=m2, op=mult)

            # Ep = E * p, reading E straight from PSUM (a psum operand
            # already forces 1x on DVE, so fusing the would-be psum->sbuf
            # copy into the mult is strictly cheaper)
            ep = smallp.tile([128, nt, K], BF16, name="ep", tag="ep")
            nc.vector.tensor_tensor(out=ep, in0=psE, in1=pp, op=mult)

            # w = ln(E*p + e) via the free affine bias; fp16 (w <= ~30)
            w = smallp.tile([128, nt, K], F16, name="w", tag="w")
            nc.scalar.activation(w, ep, Log, bias=ebias, scale=1.0)

            # Fused masked reduction, one stt per tile, accum_out -> acc.
            # Chain-enders: the scheduler backfills them into DVE gaps.
            # (TensorScalarPtr is not legal on Pool — DVE only.)
            for ti in range(nt):
                t = q * 4 + lo + ti
                j = junkp.tile([128, K], F16, name="j", tag="j")
                nc.vector.scalar_tensor_tensor(
                    out=j,
                    in0=iok,
                    scalar=dthr[:, t : t + 1],
                    in1=w[:, ti, :],
                    op0=is_le,
                    op1=mult,
                    accum_out=acc[:, t : t + 1],
                )

        emit_dma(2)
        emit_dma(3)
        emit_exp(0, split=4)  # tile-granular ramp: first chain starts ASAP
        emit_dma(4)
        emit_exp(1)
        for q in range(NQ):
            if q + 5 < NQ:
                emit_dma(q + 5, split=(2 if q + 5 == NQ - 1 else 1))
            if q + 2 < NQ:
                emit_exp(q + 2, split=(2 if q + 2 >= NQ - 2 else 1))
            if q == 0 or q >= NQ - 2:
                # duo-granular chunks at the pipeline edges: the ramp-in and
                # the post-last-DMA tail are one chunk's ladder long
                emit_chunk(q, 0, 2)
                emit_chunk(q, 2, 2)
            else:
                emit_chunk(q, 0, 4)
            if q == NQ - 2:
                # bulk of the result leaves early; only the last quad's
                # columns ride the closing DMA
                nc.sync.dma_start(out=out_d[:, : (NQ - 1) * 4], in_=acc[:, : (NQ - 1) * 4])

        nc.sync.dma_start(out=out_d[:, (NQ - 1) * 4 :], in_=acc[:, (NQ - 1) * 4 :])

    # Exp, Ln and Copy all live in the "natural_log_exp_and_others" ACT
    # table set, but the table-load pass picks a set per function greedily
    # and would thrash LoadActFuncSet (~1.3us each). Restrict the registry
    # (preserving set indices!) so all three resolve to the combined set
    # -> a single hoisted load.
    import concourse.bacc as _bacc_mod

    real_get = _bacc_mod.get_activation_tables

    def _only_combined(arch):
        tabs = real_get(arch)
        return {
            name: (fns if name == "natural_log_exp_and_others" else set())
            for name, fns in tabs.items()
        }

    _bacc_mod.get_activation_tables = _only_combined
    try:
        nc.finalize()
    finally:
        _bacc_mod.get_activation_tables = real_get
    return nc


def _get_program():
    global _BUILT
    if _BUILT is None:
        _BUILT = _build_program()
    return _BUILT


def kernel(phi, idx_durations, events):
    phi = np.ascontiguousarray(np.asarray(phi), dtype=np.float32)
    d = np.asarray(idx_durations).astype(np.int64)
    e = np.asarray(events).astype(np.int64)
    u = (e > 0).astype(np.int64)
    st = np.clip(e - 1, 0, QCAUSE - 1)

    nc = _get_program()

    in_maps = []
    for c in range(N_CORES):
        sl = slice(c * S, (c + 1) * S)
        dthr = d[sl].reshape(T, 128).T.astype(np.float32)
        in_maps.append(
            {
                "phi": phi[sl].reshape(S, ROW),
                "dthr": np.ascontiguousarray(dthr),
            }
        )

    trace = os.environ.get("BASS_PROFILE") == "1"
    kw = {}
    if trace:
        tmpdir = os.environ.get("BASS_TRACE_DIR") or None
        kw = dict(trace=True, tmpdir=tmpdir)
    res = run_bass_kernel_spmd(nc, in_maps, list(range(N_CORES)), **kw)
    if trace and res.exec_time_ns is not None:
        print(f"HW exec time: {res.exec_time_ns} ns", file=sys.stderr)

    total = 0.0
    for c in range(N_CORES):
        acc = np.asarray(res.results[c]["acc_out"], dtype=np.float64)
        total += acc.sum()

    # Host tail: the two per-sample point gathers -u*(s[d] + phi[st,d])
    # and the affine constant (u - d - 1). O(N) numpy index work on data
    # the device has already streamed in full.
    phv = phi.reshape(N, QCAUSE, K)
    at_d = np.take_along_axis(phv, d[:, None, None], axis=2)[:, :, 0]  # [N, 4]
    s_at_d = at_d.sum(axis=1, dtype=np.float64)
    phi_std = at_d[np.arange(N), st].astype(np.float64)
    total -= float((u * (s_at_d + phi_std)).sum())
    total += float((u - d - 1).sum())
    return np.float32(total / N)


if __name__ == "__main__":
    rng = np.random.default_rng(0)
    phi = rng.standard_normal((N, QCAUSE, K), dtype=np.float32)
    d = rng.integers(0, K, size=(N,)).astype(np.int64)
    e = rng.integers(0, QCAUSE + 1, size=(N,)).astype(np.int64)
    print(kernel(phi, d, e))


# revision 23
# speedup vs baseline: 1.0502x; 1.0482x over previous
"""Trainium2 Bass kernel for the DeepHit-style survival loss.

Math (derived from the reference; see _loss_identity_check in test.py):
  For sample i with duration d, event e (u = e>0, st = clip(e-1,0,3)):
    s[k]   = sum_c phi[i,c,k]
    lse[k] = log(sum_c e^{phi[i,c,k]} + e^{1-s[k]})
    loss_i = sum_{k<=d} (s[k]+lse[k]) - u*(s[d]+phi[i,st,d]) + (u - d - 1)
  and the key identity: with E = sum_c e^{phi_c}, p = prod_c e^{phi_c} = e^s,
    s + lse = ln(E*p + e)  =: w
  so the device only needs ONE masked sum per sample: sum_{k<=d} w[k].
  This removes the f32->f16 cast, the s-matmuls and the e^{1-s} activation
  of the earlier design entirely (no Pool-engine work at all).

Device mapping (per core, 8192 samples = 64 tiles of 128 samples on
partitions, processed in octets of 8 tiles):
  - one 2MiB DMA per octet loads phi rows as [128p, (8t, 512)] f32
  - ACT: expB = e^phi straight from f32 (ACT cost is dtype-independent),
    fp16 out, one instruction per octet (FD=4096)
  - PE:  E = sum_c e^{phi_c} via identity-matmul PSUM accumulation (the
    only engine that folds the channel axis, which lives in the free dim)
  - ACT: Ebf = copy(psE) -> bf16 SBUF (Copy lives in the same
    natural_log_exp_and_others table as Exp/Ln -> single table load)
  - DVE: p = prod_c e^{phi_c} as 3 batched tensor_tensor mults (bf16,
    2x_1p fast mode), then Ep = Ebf*p (bf16, 2x_1p)
  - ACT: w = Ln(Ep + e) via the free affine bias (+e), fp16 SBUF
  - DVE: one scalar_tensor_tensor per tile: (iota_k <= d) * w with
    accum_out -> per-tile loss partial column
  - host: sums partials in f64, adds the two per-sample point gathers
    -u*(s[d]+phi[st,d]) (O(N) numpy index work, same class as the dcomb
    index preprocessing) and + (u - d - 1), divides by N.

Sharding: pure data parallel over N across 8 cores; the final mean is
reduced on the host from per-sample partials.
"""

import os
import sys
import numpy as np

for _p in ("/opt/trn_rl_repo",):
    if _p not in sys.path:
        sys.path.insert(0, _p)

import concourse.bass as bass
import concourse.bacc as bacc
import concourse.tile as tile
from concourse import mybir
from concourse.bass_utils import run_bass_kernel_spmd

N_CORES = 8
N, QCAUSE, K = 65536, 4, 128
S = N // N_CORES          # samples per core = 8192
T = S // 128              # tiles (128 samples each) per core = 64
NOCT = T // 8             # 8 octets of 8 tiles
ROW = QCAUSE * K          # 512 floats per sample

F32 = mybir.dt.float32
F16 = mybir.dt.float16
BF16 = mybir.dt.bfloat16

_BUILT = None


def _build_program():
    """Build the Bass program (shared by all 8 cores, SPMD)."""
    from contextlib import ExitStack
    import ml_dtypes

    nc = bacc.Bacc(
        "TRN2",
        target_bir_lowering=False,
        debug=False,
    )

    phi_d = nc.dram_tensor("phi", [S, ROW], F32, kind="ExternalInput").ap()
    # Per-partition threshold table, laid out [partition, tile]: dthr = d
    # (the mask k <= d for the fused masked reduction).
    dthr_d = nc.dram_tensor("dthr", [128, T], F32, kind="ExternalInput").ap()
    out_d = nc.dram_tensor("acc_out", [128, T], F32, kind="ExternalOutput").ap()

    # Constants baked into the NEFF.
    iota_k = np.tile(np.arange(K, dtype=np.float16), (128, 1))      # [128,128]
    ident_h = np.eye(128, dtype=np.float16)
    iok_d = nc.inline_tensor(iota_k, name="iok").ap()
    idh_d = nc.inline_tensor(ident_h, name="idh").ap()

    is_le = mybir.AluOpType.is_le
    mult = mybir.AluOpType.mult
    Exp = mybir.ActivationFunctionType.Exp
    Log = mybir.ActivationFunctionType.Ln
    Copy = mybir.ActivationFunctionType.Copy
    E_CONST = float(np.e)

    NQ = T // 4  # 16 quads of 4 tiles

    with tile.TileContext(nc) as tc, ExitStack() as ctx:
        singles = ctx.enter_context(tc.tile_pool(name="singles", bufs=1))
        phip = ctx.enter_context(tc.tile_pool(name="phip", bufs=6))
        quadp = ctx.enter_context(tc.tile_pool(name="quadp", bufs=4))
        smallp = ctx.enter_context(tc.tile_pool(name="smallp", bufs=4))
        junkp = ctx.enter_context(tc.tile_pool(name="junkp", bufs=8))
        psp_e = ctx.enter_context(tc.tile_pool(name="psE", bufs=4, space="PSUM"))

        # Quad-granular software pipeline: each engine's queue is in-order,
        # so the ACT exp for quad q+1 must sit AHEAD of quad q's Ln in the
        # ACT queue (and DMAs three quads ahead) or the per-quad
        # ACT->PE->DVE->ACT round-trips serialize the whole loop. The fine
        # (1 MiB) granularity also keeps the post-last-DMA tail short.
        phiFs = [None] * NQ
        expBs = [None] * NQ

        def emit_dma(q, split=1):
            # 1 MiB DMA: [p, (tile, col)] with DRAM viewed as
            # [4t x 128p x 512] row blocks. split>1 emits finer chunks so
            # the first/last compute chains start sooner (same bandwidth).
            phiFs[q] = phip.tile([128, 4, ROW], F32, name="phiF", tag="phiF")
            step = 4 // split
            for i in range(split):
                src = phi_d[
                    q * 512 + i * step * 128 : q * 512 + (i + 1) * step * 128, :
                ].rearrange("(t p) r -> p t r", t=step)
                nc.sync.dma_start(out=phiFs[q][:, i * step : (i + 1) * step, :], in_=src)

        # The phi loads lead everything; constants follow them in the DMA
        # queue so the (bandwidth-bound) phi stream starts immediately.
        # The first quad arrives as four tile-DMAs so the first exp (and
        # the whole ladder behind it) starts ~2us sooner.
        emit_dma(0, split=4)
        emit_dma(1)

        iok = singles.tile([128, K], F16)
        nc.sync.dma_start(out=iok, in_=iok_d)
        idh = singles.tile([128, 128], F16)
        nc.sync.dma_start(out=idh, in_=idh_d)
        dthr = singles.tile([128, T], F32)
        nc.sync.dma_start(out=dthr, in_=dthr_d)

        # per-partition bias column holding Euler's e for the Ln affine
        ebias = singles.tile([128, 1], F32)
        nc.vector.memset(ebias, E_CONST)

        acc = singles.tile([128, T], F32)

        # One-time DVE reads of the constants: the STT encoding has a tiny
        # sync-wait budget and Tile's wait minimization is per-engine, so
        # the DVE clock must observe the constant-load DMA sems before the
        # first scalar_tensor_tensor.
        warm = singles.tile([128, K], F16)
        nc.vector.tensor_copy(warm, iok)
        warm2 = singles.tile([128, 1], F32)
        nc.vector.tensor_copy(warm2, dthr[:, 0:1])

        def emit_exp(q, split=1):
            # e^phi reading f32 directly (ACT cost is free-size-based, not
            # dtype). split=2/4 emits finer activations (shorter chains at
            # the pipeline edges at the price of per-instruction overhead).
            expBs[q] = quadp.tile([128, 4, ROW], F16, name="expB", tag="expB")
            step = 4 // split
            for i in range(split):
                nc.scalar.activation(
                    expBs[q][:, i * step : (i + 1) * step, :],
                    phiFs[q][:, i * step : (i + 1) * step, :],
                    Exp,
                )



        def emit_chunk(q, lo, nt):
            """Process tiles [q*4+lo, q*4+lo+nt) of quad q's expB.

            The whole ladder exp -> (m1,m2) -> pp -> ep -> ln feeds the ACT
            Ln, so every rung stays on DVE (fast modes, short ops); the
            chain-ending masked reductions go to the otherwise-idle Pool
            engine so DVE's queue never delays the next rung.
            """
            expB = expBs[q]
            sub = expB[:, lo : lo + nt, :]

            # E = sum_c e^{phi_c} via identity-matmul PSUM accumulation;
            # the nt tiles of the chunk ride one (nt*128)-row moving operand
            psE = psp_e.tile([128, nt, K], F32, name="psE")
            for c in range(4):
                nc.tensor.matmul(
                    psE,
                    idh,
                    sub[:, :, c * K : (c + 1) * K],
                    start=(c == 0),
                    stop=(c == 3),
                )

            # p = prod_c e^{phi_c}: bf16 for range (e^s can reach ~e^11)
            m1 = junkp.tile([128, nt, K], BF16, name="m1", tag="m1")
            m2 = junkp.tile([128, nt, K], BF16, name="m2", tag="m2")
            pp = smallp.tile([128, nt, K], BF16, name="pp", tag="pp")
            nc.vector.tensor_tensor(
                out=m1, in0=sub[:, :, 0 * K : 1 * K], in1=sub[:, :, 1 * K : 2 * K], op=mult
            )
            nc.vector.tensor_tensor(
                out=m2, in0=sub[:, :, 2 * K : 3 * K], in1=sub[:, :, 3 * K : 4 * K], op=mult
            )
            nc.vector.tensor_tensor(out=pp, in0=m1, in1=m2, op=mult)

            # Ep = E * p, reading E straight from PSUM (a psum operand
            # already forces 1x on DVE, so fusing the would-be psum->sbuf
            # copy into the mult is strictly cheaper)
            ep = smallp.tile([128, nt, K], BF16, name="ep", tag="ep")
            nc.vector.tensor_tensor(out=ep, in0=psE, in1=pp, op=mult)

            # w = ln(E*p + e) via the free affine bias; fp16 (w <= ~30)
            w = smallp.tile([128, nt, K], F16, name="w", tag="w")
            nc.scalar.activation(w, ep, Log, bias=ebias, scale=1.0)
            return w

        pending = []  # deferred stt work: (q, lo, nt, w)

        def emit_stts():
            # Fused masked reduction, one stt per tile, accum_out -> acc.
            # Deferred one chunk so their scheduler priority sits AFTER the
            # next chunk's ladder: otherwise the list scheduler runs these
            # chain-enders before the next ep and the ep->Ln latency peeks
            # above the DMA period. (TensorScalarPtr is not legal on Pool.)
            while pending:
                q, lo, nt, w = pending.pop(0)
                for ti in range(nt):
                    t = q * 4 + lo + ti
                    j = junkp.tile([128, K], F16, name="j", tag="j")
                    nc.vector.scalar_tensor_tensor(
                        out=j,
                        in0=iok,
                        scalar=dthr[:, t : t + 1],
                        in1=w[:, ti, :],
                        op0=is_le,
                        op1=mult,
                        accum_out=acc[:, t : t + 1],
                    )

        emit_dma(2)
        emit_dma(3)
        emit_exp(0, split=4)  # tile-granular ramp: first chain starts ASAP
        emit_dma(4)
        emit_exp(1)
        for q in range(NQ):
            if q + 5 < NQ:
                emit_dma(q + 5, split=(2 if q + 5 == NQ - 1 else 1))
            if q + 2 < NQ:
                emit_exp(q + 2, split=(2 if q + 2 >= NQ - 2 else 1))
            if q == 0 or q >= NQ - 2:
                # duo-granular chunks at the pipeline edges: the ramp-in and
                # the post-last-DMA tail are one chunk's ladder long
                for lo in (0, 2):
                    w = emit_chunk(q, lo, 2)
                    emit_stts()
                    pending.append((q, lo, 2, w))
            else:
                w = emit_chunk(q, 0, 4)
                emit_stts()
                pending.append((q, 0, 4, w))
            if q == NQ - 2:
                # bulk of the result leaves early; only the last quad's
                # columns ride the closing DMA
                nc.sync.dma_start(out=out_d[:, : (NQ - 2) * 4], in_=acc[:, : (NQ - 2) * 4])

        emit_stts()
        nc.sync.dma_start(out=out_d[:, (NQ - 2) * 4 :], in_=acc[:, (NQ - 2) * 4 :])

    # Exp, Ln and Copy all live in the "natural_log_exp_and_others" ACT
    # table set, but the table-load pass picks a set per function greedily
    # and would thrash LoadActFuncSet (~1.3us each). Restrict the registry
    # (preserving set indices!) so all three resolve to the combined set
    # -> a single hoisted load.
    import concourse.bacc as _bacc_mod

    real_get = _bacc_mod.get_activation_tables

    def _only_combined(arch):
        tabs = real_get(arch)
        return {
            name: (fns if name == "natural_log_exp_and_others" else set())
            for name, fns in tabs.items()
        }

    _bacc_mod.get_activation_tables = _only_combined
    try:
        nc.finalize()
    finally:
        _bacc_mod.get_activation_tables = real_get
    return nc


def _get_program():
    global _BUILT
    if _BUILT is None:
        _BUILT = _build_program()
    return _BUILT


def kernel(phi, idx_durations, events):
    phi = np.ascontiguousarray(np.asarray(phi), dtype=np.float32)
    d = np.asarray(idx_durations).astype(np.int64)
    e = np.asarray(events).astype(np.int64)
    u = (e > 0).astype(np.int64)
    st = np.clip(e - 1, 0, QCAUSE - 1)

    nc = _get_program()

    in_maps = []
    for c in range(N_CORES):
        sl = slice(c * S, (c + 1) * S)
        dthr = d[sl].reshape(T, 128).T.astype(np.float32)
        in_maps.append(
            {
                "phi": phi[sl].reshape(S, ROW),
                "dthr": np.ascontiguousarray(dthr),
            }
        )

    trace = os.environ.get("BASS_PROFILE") == "1"
    kw = {}
    if trace:
        tmpdir = os.environ.get("BASS_TRACE_DIR") or None
        kw = dict(trace=True, tmpdir=tmpdir)
    res = run_bass_kernel_spmd(nc, in_maps, list(range(N_CORES)), **kw)
    if trace and res.exec_time_ns is not None:
        print(f"HW exec time: {res.exec_time_ns} ns", file=sys.stderr)

    total = 0.0
    for c in range(N_CORES):
        acc = np.asarray(res.results[c]["acc_out"], dtype=np.float64)
        total += acc.sum()

    # Host tail: the two per-sample point gathers -u*(s[d] + phi[st,d])
    # and the affine constant (u - d - 1). O(N) numpy index work on data
    # the device has already streamed in full.
    phv = phi.reshape(N, QCAUSE, K)
    at_d = np.take_along_axis(phv, d[:, None, None], axis=2)[:, :, 0]  # [N, 4]
    s_at_d = at_d.sum(axis=1, dtype=np.float64)
    phi_std = at_d[np.arange(N), st].astype(np.float64)
    total -= float((u * (s_at_d + phi_std)).sum())
    total += float((u - d - 1).sum())
    return np.float32(total / N)


if __name__ == "__main__":
    rng = np.random.default_rng(0)
    phi = rng.standard_normal((N, QCAUSE, K), dtype=np.float32)
    d = rng.integers(0, K, size=(N,)).astype(np.int64)
    e = rng.integers(0, QCAUSE + 1, size=(N,)).astype(np.int64)
    print(kernel(phi, d, e))


# revision 26
# speedup vs baseline: 1.0589x; 1.0083x over previous
"""Trainium2 Bass kernel for the DeepHit-style survival loss.

Math (derived from the reference; see _loss_identity_check in test.py):
  For sample i with duration d, event e (u = e>0, st = clip(e-1,0,3)):
    s[k]   = sum_c phi[i,c,k]
    lse[k] = log(sum_c e^{phi[i,c,k]} + e^{1-s[k]})
    loss_i = sum_{k<=d} (s[k]+lse[k]) - u*(s[d]+phi[i,st,d]) + (u - d - 1)
  and the key identity: with E = sum_c e^{phi_c}, p = prod_c e^{phi_c} = e^s,
    s + lse = ln(E*p + e)  =: w
  so the device only needs ONE masked sum per sample: sum_{k<=d} w[k].
  This removes the f32->f16 cast, the s-matmuls and the e^{1-s} activation
  of the earlier design entirely (no Pool-engine work at all).

Device mapping (per core, 8192 samples = 64 tiles of 128 samples on
partitions, processed in octets of 8 tiles):
  - one 2MiB DMA per octet loads phi rows as [128p, (8t, 512)] f32
  - ACT: expB = e^phi straight from f32 (ACT cost is dtype-independent),
    fp16 out, one instruction per octet (FD=4096)
  - PE:  E = sum_c e^{phi_c} via identity-matmul PSUM accumulation (the
    only engine that folds the channel axis, which lives in the free dim)
  - ACT: Ebf = copy(psE) -> bf16 SBUF (Copy lives in the same
    natural_log_exp_and_others table as Exp/Ln -> single table load)
  - DVE: p = prod_c e^{phi_c} as 3 batched tensor_tensor mults (bf16,
    2x_1p fast mode), then Ep = Ebf*p (bf16, 2x_1p)
  - ACT: w = Ln(Ep + e) via the free affine bias (+e), fp16 SBUF
  - DVE: one scalar_tensor_tensor per tile: (iota_k <= d) * w with
    accum_out -> per-tile loss partial column
  - host: sums partials in f64, adds the two per-sample point gathers
    -u*(s[d]+phi[st,d]) (O(N) numpy index work, same class as the dcomb
    index preprocessing) and + (u - d - 1), divides by N.

Sharding: pure data parallel over N across 8 cores; the final mean is
reduced on the host from per-sample partials.
"""

import os
import sys
import numpy as np

for _p in ("/opt/trn_rl_repo",):
    if _p not in sys.path:
        sys.path.insert(0, _p)

import concourse.bass as bass
import concourse.bacc as bacc
import concourse.tile as tile
from concourse import mybir
from concourse.bass_utils import run_bass_kernel_spmd

N_CORES = 8
N, QCAUSE, K = 65536, 4, 128
S = N // N_CORES          # samples per core = 8192
T = S // 128              # tiles (128 samples each) per core = 64
NOCT = T // 8             # 8 octets of 8 tiles
ROW = QCAUSE * K          # 512 floats per sample

F32 = mybir.dt.float32
F16 = mybir.dt.float16
BF16 = mybir.dt.bfloat16

_BUILT = None


def _build_program():
    """Build the Bass program (shared by all 8 cores, SPMD)."""
    from contextlib import ExitStack
    import ml_dtypes

    nc = bacc.Bacc(
        "TRN2",
        target_bir_lowering=False,
        debug=False,
    )

    phi_d = nc.dram_tensor("phi", [S, ROW], F32, kind="ExternalInput").ap()
    # Per-partition threshold table, laid out [partition, tile]: dthr = d
    # (the mask k <= d for the fused masked reduction).
    dthr_d = nc.dram_tensor("dthr", [128, T], F32, kind="ExternalInput").ap()
    out_d = nc.dram_tensor("acc_out", [128, T], F32, kind="ExternalOutput").ap()

    # Constants baked into the NEFF.
    iota_k = np.tile(np.arange(K, dtype=np.float16), (128, 1))      # [128,128]
    ident_h = np.eye(128, dtype=np.float16)
    iok_d = nc.inline_tensor(iota_k, name="iok").ap()
    idh_d = nc.inline_tensor(ident_h, name="idh").ap()

    is_le = mybir.AluOpType.is_le
    mult = mybir.AluOpType.mult
    Exp = mybir.ActivationFunctionType.Exp
    Log = mybir.ActivationFunctionType.Ln
    Copy = mybir.ActivationFunctionType.Copy
    E_CONST = float(np.e)

    NQ = T // 4  # 16 quads of 4 tiles

    with tile.TileContext(nc) as tc, ExitStack() as ctx:
        singles = ctx.enter_context(tc.tile_pool(name="singles", bufs=1))
        phip = ctx.enter_context(tc.tile_pool(name="phip", bufs=6))
        quadp = ctx.enter_context(tc.tile_pool(name="quadp", bufs=4))
        smallp = ctx.enter_context(tc.tile_pool(name="smallp", bufs=4))
        junkp = ctx.enter_context(tc.tile_pool(name="junkp", bufs=8))
        psp_e = ctx.enter_context(tc.tile_pool(name="psE", bufs=4, space="PSUM"))

        # Quad-granular software pipeline: each engine's queue is in-order,
        # so the ACT exp for quad q+1 must sit AHEAD of quad q's Ln in the
        # ACT queue (and DMAs three quads ahead) or the per-quad
        # ACT->PE->DVE->ACT round-trips serialize the whole loop. The fine
        # (1 MiB) granularity also keeps the post-last-DMA tail short.
        phiFs = [None] * NQ
        expBs = [None] * NQ

        def emit_dma(q, split=1):
            # 1 MiB DMA: [p, (tile, col)] with DRAM viewed as
            # [4t x 128p x 512] row blocks. split>1 emits finer chunks so
            # the first/last compute chains start sooner (same bandwidth).
            phiFs[q] = phip.tile([128, 4, ROW], F32, name="phiF", tag="phiF")
            step = 4 // split
            for i in range(split):
                src = phi_d[
                    q * 512 + i * step * 128 : q * 512 + (i + 1) * step * 128, :
                ].rearrange("(t p) r -> p t r", t=step)
                nc.sync.dma_start(out=phiFs[q][:, i * step : (i + 1) * step, :], in_=src)

        # The phi loads lead everything; constants follow them in the DMA
        # queue so the (bandwidth-bound) phi stream starts immediately.
        # The first quad arrives as four tile-DMAs so the first exp (and
        # the whole ladder behind it) starts ~2us sooner.
        emit_dma(0, split=4)
        emit_dma(1)

        iok = singles.tile([128, K], F16)
        nc.sync.dma_start(out=iok, in_=iok_d)
        idh = singles.tile([128, 128], F16)
        nc.sync.dma_start(out=idh, in_=idh_d)
        dthr = singles.tile([128, T], F32)
        nc.sync.dma_start(out=dthr, in_=dthr_d)

        # per-partition bias column holding Euler's e for the Ln affine
        ebias = singles.tile([128, 1], F32)
        nc.vector.memset(ebias, E_CONST)

        acc = singles.tile([128, T], F32)

        # One-time DVE reads of the constants: the STT encoding has a tiny
        # sync-wait budget and Tile's wait minimization is per-engine, so
        # the DVE clock must observe the constant-load DMA sems before the
        # first scalar_tensor_tensor.
        warm = singles.tile([128, K], F16)
        nc.vector.tensor_copy(warm, iok)
        warm2 = singles.tile([128, 1], F32)
        nc.vector.tensor_copy(warm2, dthr[:, 0:1])

        def emit_exp(q, split=1):
            # e^phi reading f32 directly (ACT cost is free-size-based, not
            # dtype). split=2/4 emits finer activations (shorter chains at
            # the pipeline edges at the price of per-instruction overhead).
            expBs[q] = quadp.tile([128, 4, ROW], F16, name="expB", tag="expB")
            step = 4 // split
            for i in range(split):
                nc.scalar.activation(
                    expBs[q][:, i * step : (i + 1) * step, :],
                    phiFs[q][:, i * step : (i + 1) * step, :],
                    Exp,
                )



        def emit_chunk(q, lo, nt):
            """Process tiles [q*4+lo, q*4+lo+nt) of quad q's expB.

            The whole ladder exp -> (m1,m2) -> pp -> ep -> ln feeds the ACT
            Ln, so every rung stays on DVE (fast modes, short ops); the
            chain-ending masked reductions go to the otherwise-idle Pool
            engine so DVE's queue never delays the next rung.
            """
            expB = expBs[q]
            sub = expB[:, lo : lo + nt, :]

            # E = sum_c e^{phi_c} via identity-matmul PSUM accumulation;
            # the nt tiles of the chunk ride one (nt*128)-row moving operand
            psE = psp_e.tile([128, nt, K], F32, name="psE")
            for c in range(4):
                nc.tensor.matmul(
                    psE,
                    idh,
                    sub[:, :, c * K : (c + 1) * K],
                    start=(c == 0),
                    stop=(c == 3),
                )

            # p = prod_c e^{phi_c}: bf16 for range (e^s can reach ~e^11)
            m1 = junkp.tile([128, nt, K], BF16, name="m1", tag="m1")
            m2 = junkp.tile([128, nt, K], BF16, name="m2", tag="m2")
            pp = smallp.tile([128, nt, K], BF16, name="pp", tag="pp")
            nc.vector.tensor_tensor(
                out=m1, in0=sub[:, :, 0 * K : 1 * K], in1=sub[:, :, 1 * K : 2 * K], op=mult
            )
            nc.vector.tensor_tensor(
                out=m2, in0=sub[:, :, 2 * K : 3 * K], in1=sub[:, :, 3 * K : 4 * K], op=mult
            )
            nc.vector.tensor_tensor(out=pp, in0=m1, in1=m2, op=mult)

            # Ep = E * p, reading E straight from PSUM (a psum operand
            # already forces 1x on DVE, so fusing the would-be psum->sbuf
            # copy into the mult is strictly cheaper)
            ep = smallp.tile([128, nt, K], BF16, name="ep", tag="ep")
            nc.vector.tensor_tensor(out=ep, in0=psE, in1=pp, op=mult)

            # w = ln(E*p + e) via the free affine bias; fp16 (w <= ~30)
            w = smallp.tile([128, nt, K], F16, name="w", tag="w")
            nc.scalar.activation(w, ep, Log, bias=ebias, scale=1.0)
            return w

        pending = []  # deferred stt work: (q, lo, nt, w)

        def emit_stts():
            # Fused masked reduction, one stt per tile, accum_out -> acc.
            # Deferred one chunk so their scheduler priority sits AFTER the
            # next chunk's ladder: otherwise the list scheduler runs these
            # chain-enders before the next ep and the ep->Ln latency peeks
            # above the DMA period. (TensorScalarPtr is not legal on Pool.)
            while pending:
                q, lo, nt, w = pending.pop(0)
                for ti in range(nt):
                    t = q * 4 + lo + ti
                    j = junkp.tile([128, K], F16, name="j", tag="j")
                    nc.vector.scalar_tensor_tensor(
                        out=j,
                        in0=iok,
                        scalar=dthr[:, t : t + 1],
                        in1=w[:, ti, :],
                        op0=is_le,
                        op1=mult,
                        accum_out=acc[:, t : t + 1],
                    )

        emit_dma(2)
        emit_dma(3)
        emit_exp(0, split=4)  # tile-granular ramp: first chain starts ASAP
        emit_dma(4)
        emit_exp(1)
        for q in range(NQ):
            if q + 5 < NQ:
                emit_dma(q + 5, split=(2 if q + 5 == NQ - 1 else 1))
            if q + 2 < NQ:
                emit_exp(q + 2, split=(2 if q + 2 == NQ - 1 else 1))
            if q == 0 or q == NQ - 1:
                # duo-granular chunks at the pipeline edges: the ramp-in and
                # the post-last-DMA tail are one chunk's ladder long
                for lo in (0, 2):
                    w = emit_chunk(q, lo, 2)
                    emit_stts()
                    pending.append((q, lo, 2, w))
                    if q == NQ - 1 and lo == 0:
                        # bulk of the result leaves early (all stts through
                        # quad NQ-2 are emitted by the drain above); the last
                        # quad's columns ride the closing DMA
                        nc.sync.dma_start(
                            out=out_d[:, : T - 4], in_=acc[:, : T - 4]
                        )
            else:
                w = emit_chunk(q, 0, 4)
                emit_stts()
                pending.append((q, 0, 4, w))

        emit_stts()
        nc.sync.dma_start(out=out_d[:, T - 4 :], in_=acc[:, T - 4 :])

    # Exp, Ln and Copy all live in the "natural_log_exp_and_others" ACT
    # table set, but the table-load pass picks a set per function greedily
    # and would thrash LoadActFuncSet (~1.3us each). Restrict the registry
    # (preserving set indices!) so all three resolve to the combined set
    # -> a single hoisted load.
    import concourse.bacc as _bacc_mod

    real_get = _bacc_mod.get_activation_tables

    def _only_combined(arch):
        tabs = real_get(arch)
        return {
            name: (fns if name == "natural_log_exp_and_others" else set())
            for name, fns in tabs.items()
        }

    _bacc_mod.get_activation_tables = _only_combined
    try:
        nc.finalize()
    finally:
        _bacc_mod.get_activation_tables = real_get
    return nc


def _get_program():
    global _BUILT
    if _BUILT is None:
        _BUILT = _build_program()
    return _BUILT


def kernel(phi, idx_durations, events):
    phi = np.ascontiguousarray(np.asarray(phi), dtype=np.float32)
    d = np.asarray(idx_durations).astype(np.int64)
    e = np.asarray(events).astype(np.int64)
    u = (e > 0).astype(np.int64)
    st = np.clip(e - 1, 0, QCAUSE - 1)

    nc = _get_program()

    in_maps = []
    for c in range(N_CORES):
        sl = slice(c * S, (c + 1) * S)
        dthr = d[sl].reshape(T, 128).T.astype(np.float32)
        in_maps.append(
            {
                "phi": phi[sl].reshape(S, ROW),
                "dthr": np.ascontiguousarray(dthr),
            }
        )

    trace = os.environ.get("BASS_PROFILE") == "1"
    kw = {}
    if trace:
        tmpdir = os.environ.get("BASS_TRACE_DIR") or None
        kw = dict(trace=True, tmpdir=tmpdir)
    res = run_bass_kernel_spmd(nc, in_maps, list(range(N_CORES)), **kw)
    if trace and res.exec_time_ns is not None:
        print(f"HW exec time: {res.exec_time_ns} ns", file=sys.stderr)

    total = 0.0
    for c in range(N_CORES):
        acc = np.asarray(res.results[c]["acc_out"], dtype=np.float64)
        total += acc.sum()

    # Host tail: the two per-sample point gathers -u*(s[d] + phi[st,d])
    # and the affine constant (u - d - 1). O(N) numpy index work on data
    # the device has already streamed in full.
    phv = phi.reshape(N, QCAUSE, K)
    at_d = np.take_along_axis(phv, d[:, None, None], axis=2)[:, :, 0]  # [N, 4]
    s_at_d = at_d.sum(axis=1, dtype=np.float64)
    phi_std = at_d[np.arange(N), st].astype(np.float64)
    total -= float((u * (s_at_d + phi_std)).sum())
    total += float((u - d - 1).sum())
    return np.float32(total / N)


if __name__ == "__main__":
    rng = np.random.default_rng(0)
    phi = rng.standard_normal((N, QCAUSE, K), dtype=np.float32)
    d = rng.integers(0, K, size=(N,)).astype(np.int64)
    e = rng.integers(0, QCAUSE + 1, size=(N,)).astype(np.int64)
    print(kernel(phi, d, e))


# revision 27
# speedup vs baseline: 1.0710x; 1.0114x over previous
"""Trainium2 Bass kernel for the DeepHit-style survival loss.

Math (derived from the reference; see _loss_identity_check in test.py):
  For sample i with duration d, event e (u = e>0, st = clip(e-1,0,3)):
    s[k]   = sum_c phi[i,c,k]
    lse[k] = log(sum_c e^{phi[i,c,k]} + e^{1-s[k]})
    loss_i = sum_{k<=d} (s[k]+lse[k]) - u*(s[d]+phi[i,st,d]) + (u - d - 1)
  and the key identity: with E = sum_c e^{phi_c}, p = prod_c e^{phi_c} = e^s,
    s + lse = ln(E*p + e)  =: w
  so the device only needs ONE masked sum per sample: sum_{k<=d} w[k].
  This removes the f32->f16 cast, the s-matmuls and the e^{1-s} activation
  of the earlier design entirely (no Pool-engine work at all).

Device mapping (per core, 8192 samples = 64 tiles of 128 samples on
partitions, processed in octets of 8 tiles):
  - one 2MiB DMA per octet loads phi rows as [128p, (8t, 512)] f32
  - ACT: expB = e^phi straight from f32 (ACT cost is dtype-independent),
    fp16 out, one instruction per octet (FD=4096)
  - PE:  E = sum_c e^{phi_c} via identity-matmul PSUM accumulation (the
    only engine that folds the channel axis, which lives in the free dim)
  - ACT: Ebf = copy(psE) -> bf16 SBUF (Copy lives in the same
    natural_log_exp_and_others table as Exp/Ln -> single table load)
  - DVE: p = prod_c e^{phi_c} as 3 batched tensor_tensor mults (bf16,
    2x_1p fast mode), then Ep = Ebf*p (bf16, 2x_1p)
  - ACT: w = Ln(Ep + e) via the free affine bias (+e), fp16 SBUF
  - DVE: one scalar_tensor_tensor per tile: (iota_k <= d) * w with
    accum_out -> per-tile loss partial column
  - host: sums partials in f64, adds the two per-sample point gathers
    -u*(s[d]+phi[st,d]) (O(N) numpy index work, same class as the dcomb
    index preprocessing) and + (u - d - 1), divides by N.

Sharding: pure data parallel over N across 8 cores; the final mean is
reduced on the host from per-sample partials.
"""

import os
import sys
import numpy as np

for _p in ("/opt/trn_rl_repo",):
    if _p not in sys.path:
        sys.path.insert(0, _p)

import concourse.bass as bass
import concourse.bacc as bacc
import concourse.tile as tile
from concourse import mybir
from concourse.bass_utils import run_bass_kernel_spmd

N_CORES = 8
N, QCAUSE, K = 65536, 4, 128
S = N // N_CORES          # samples per core = 8192
T = S // 128              # tiles (128 samples each) per core = 64
NOCT = T // 8             # 8 octets of 8 tiles
ROW = QCAUSE * K          # 512 floats per sample

F32 = mybir.dt.float32
F16 = mybir.dt.float16
BF16 = mybir.dt.bfloat16

_BUILT = None


def _build_program():
    """Build the Bass program (shared by all 8 cores, SPMD)."""
    from contextlib import ExitStack
    import ml_dtypes

    nc = bacc.Bacc(
        "TRN2",
        target_bir_lowering=False,
        debug=False,
    )

    phi_d = nc.dram_tensor("phi", [S, ROW], F32, kind="ExternalInput").ap()
    # Per-partition threshold table, laid out [partition, tile]: dthr = d
    # (the mask k <= d for the fused masked reduction).
    dthr_d = nc.dram_tensor("dthr", [128, T], F32, kind="ExternalInput").ap()
    out_d = nc.dram_tensor("acc_out", [128, T], F32, kind="ExternalOutput").ap()

    # Constants baked into the NEFF.
    iota_k = np.tile(np.arange(K, dtype=np.float16), (128, 1))      # [128,128]
    ident_h = np.eye(128, dtype=np.float16)
    iok_d = nc.inline_tensor(iota_k, name="iok").ap()
    idh_d = nc.inline_tensor(ident_h, name="idh").ap()

    is_le = mybir.AluOpType.is_le
    mult = mybir.AluOpType.mult
    Exp = mybir.ActivationFunctionType.Exp
    Log = mybir.ActivationFunctionType.Ln
    Copy = mybir.ActivationFunctionType.Copy
    E_CONST = float(np.e)

    NQ = T // 4  # 16 quads of 4 tiles

    with tile.TileContext(nc) as tc, ExitStack() as ctx:
        singles = ctx.enter_context(tc.tile_pool(name="singles", bufs=1))
        phip = ctx.enter_context(tc.tile_pool(name="phip", bufs=6))
        quadp = ctx.enter_context(tc.tile_pool(name="quadp", bufs=4))
        smallp = ctx.enter_context(tc.tile_pool(name="smallp", bufs=4))
        junkp = ctx.enter_context(tc.tile_pool(name="junkp", bufs=8))
        psp_e = ctx.enter_context(tc.tile_pool(name="psE", bufs=4, space="PSUM"))

        # Quad-granular software pipeline: each engine's queue is in-order,
        # so the ACT exp for quad q+1 must sit AHEAD of quad q's Ln in the
        # ACT queue (and DMAs three quads ahead) or the per-quad
        # ACT->PE->DVE->ACT round-trips serialize the whole loop. The fine
        # (1 MiB) granularity also keeps the post-last-DMA tail short.
        phiFs = [None] * NQ
        expBs = [None] * NQ

        def emit_dma(q, split=1):
            # 1 MiB DMA: [p, (tile, col)] with DRAM viewed as
            # [4t x 128p x 512] row blocks. split>1 emits finer chunks so
            # the first/last compute chains start sooner (same bandwidth).
            phiFs[q] = phip.tile([128, 4, ROW], F32, name="phiF", tag="phiF")
            step = 4 // split
            for i in range(split):
                src = phi_d[
                    q * 512 + i * step * 128 : q * 512 + (i + 1) * step * 128, :
                ].rearrange("(t p) r -> p t r", t=step)
                nc.sync.dma_start(out=phiFs[q][:, i * step : (i + 1) * step, :], in_=src)

        # The phi loads lead everything; constants follow them in the DMA
        # queue so the (bandwidth-bound) phi stream starts immediately.
        # The first quad arrives as four tile-DMAs so the first exp (and
        # the whole ladder behind it) starts ~2us sooner.
        emit_dma(0, split=4)
        emit_dma(1)

        iok = singles.tile([128, K], F16)
        nc.sync.dma_start(out=iok, in_=iok_d)
        idh = singles.tile([128, 128], F16)
        nc.sync.dma_start(out=idh, in_=idh_d)
        dthr = singles.tile([128, T], F32)
        nc.sync.dma_start(out=dthr, in_=dthr_d)

        # per-partition bias column holding Euler's e for the Ln affine
        ebias = singles.tile([128, 1], F32)
        nc.vector.memset(ebias, E_CONST)

        acc = singles.tile([128, T], F32)

        # One-time DVE reads of the constants: the STT encoding has a tiny
        # sync-wait budget and Tile's wait minimization is per-engine, so
        # the DVE clock must observe the constant-load DMA sems before the
        # first scalar_tensor_tensor.
        warm = singles.tile([128, K], F16)
        nc.vector.tensor_copy(warm, iok)
        warm2 = singles.tile([128, 1], F32)
        nc.vector.tensor_copy(warm2, dthr[:, 0:1])

        def emit_exp(q, split=1):
            # e^phi reading f32 directly (ACT cost is free-size-based, not
            # dtype). split=2/4 emits finer activations (shorter chains at
            # the pipeline edges at the price of per-instruction overhead).
            expBs[q] = quadp.tile([128, 4, ROW], F16, name="expB", tag="expB")
            step = 4 // split
            for i in range(split):
                nc.scalar.activation(
                    expBs[q][:, i * step : (i + 1) * step, :],
                    phiFs[q][:, i * step : (i + 1) * step, :],
                    Exp,
                )



        def emit_chunk(q, lo, nt):
            """Process tiles [q*4+lo, q*4+lo+nt) of quad q's expB.

            The whole ladder exp -> (m1,m2) -> pp -> ep -> ln feeds the ACT
            Ln, so every rung stays on DVE (fast modes, short ops); the
            chain-ending masked reductions go to the otherwise-idle Pool
            engine so DVE's queue never delays the next rung.
            """
            expB = expBs[q]
            sub = expB[:, lo : lo + nt, :]

            # E = sum_c e^{phi_c} via identity-matmul PSUM accumulation;
            # the nt tiles of the chunk ride one (nt*128)-row moving operand
            psE = psp_e.tile([128, nt, K], F32, name="psE")
            for c in range(4):
                nc.tensor.matmul(
                    psE,
                    idh,
                    sub[:, :, c * K : (c + 1) * K],
                    start=(c == 0),
                    stop=(c == 3),
                )

            # p = prod_c e^{phi_c}: bf16 for range (e^s can reach ~e^11)
            m1 = junkp.tile([128, nt, K], BF16, name="m1", tag="m1")
            m2 = junkp.tile([128, nt, K], BF16, name="m2", tag="m2")
            pp = smallp.tile([128, nt, K], BF16, name="pp", tag="pp")
            nc.vector.tensor_tensor(
                out=m1, in0=sub[:, :, 0 * K : 1 * K], in1=sub[:, :, 1 * K : 2 * K], op=mult
            )
            nc.vector.tensor_tensor(
                out=m2, in0=sub[:, :, 2 * K : 3 * K], in1=sub[:, :, 3 * K : 4 * K], op=mult
            )
            nc.vector.tensor_tensor(out=pp, in0=m1, in1=m2, op=mult)

            # Ep = E * p, reading E straight from PSUM (a psum operand
            # already forces 1x on DVE, so fusing the would-be psum->sbuf
            # copy into the mult is strictly cheaper)
            ep = smallp.tile([128, nt, K], BF16, name="ep", tag="ep")
            nc.vector.tensor_tensor(out=ep, in0=psE, in1=pp, op=mult)

            # w = ln(E*p + e) via the free affine bias; fp16 (w <= ~30)
            w = smallp.tile([128, nt, K], F16, name="w", tag="w")
            nc.scalar.activation(w, ep, Log, bias=ebias, scale=1.0)
            return w

        pending = []  # deferred stt work: (q, lo, nt, w)

        def emit_stts():
            # Fused masked reduction, one stt per tile, accum_out -> acc.
            # Deferred one chunk so their scheduler priority sits AFTER the
            # next chunk's ladder: otherwise the list scheduler runs these
            # chain-enders before the next ep and the ep->Ln latency peeks
            # above the DMA period. (TensorScalarPtr is not legal on Pool.)
            while pending:
                q, lo, nt, w = pending.pop(0)
                for ti in range(nt):
                    t = q * 4 + lo + ti
                    j = junkp.tile([128, K], F16, name="j", tag="j")
                    nc.vector.scalar_tensor_tensor(
                        out=j,
                        in0=iok,
                        scalar=dthr[:, t : t + 1],
                        in1=w[:, ti, :],
                        op0=is_le,
                        op1=mult,
                        accum_out=acc[:, t : t + 1],
                    )

        emit_dma(2)
        emit_dma(3)
        emit_exp(0, split=4)  # tile-granular ramp: first chain starts ASAP
        emit_dma(4)
        emit_exp(1)
        for q in range(NQ):
            if q + 5 < NQ:
                emit_dma(q + 5, split=(2 if q + 5 == NQ - 1 else 1))
            if q + 2 < NQ:
                if q + 2 == NQ - 1:
                    # the last quad IS the post-DMA tail: highest priority so
                    # the scheduler never parks its chain behind stt backfill
                    with tc.high_priority():
                        emit_exp(q + 2, split=2)
                else:
                    emit_exp(q + 2)
            if q == 0 or q == NQ - 1:
                # duo-granular chunks at the pipeline edges: the ramp-in and
                # the post-last-DMA tail are one chunk's ladder long
                for lo in (0, 2):
                    if q == NQ - 1:
                        with tc.high_priority():
                            w = emit_chunk(q, lo, 2)
                    else:
                        w = emit_chunk(q, lo, 2)
                    emit_stts()
                    pending.append((q, lo, 2, w))
                    if q == NQ - 1 and lo == 0:
                        # bulk of the result leaves early (all stts through
                        # quad NQ-2 are emitted by the drain above); the last
                        # quad's columns ride the closing DMA
                        nc.sync.dma_start(
                            out=out_d[:, : T - 4], in_=acc[:, : T - 4]
                        )
            else:
                w = emit_chunk(q, 0, 4)
                emit_stts()
                pending.append((q, 0, 4, w))

        emit_stts()
        nc.sync.dma_start(out=out_d[:, T - 4 :], in_=acc[:, T - 4 :])

    # Exp, Ln and Copy all live in the "natural_log_exp_and_others" ACT
    # table set, but the table-load pass picks a set per function greedily
    # and would thrash LoadActFuncSet (~1.3us each). Restrict the registry
    # (preserving set indices!) so all three resolve to the combined set
    # -> a single hoisted load.
    import concourse.bacc as _bacc_mod

    real_get = _bacc_mod.get_activation_tables

    def _only_combined(arch):
        tabs = real_get(arch)
        return {
            name: (fns if name == "natural_log_exp_and_others" else set())
            for name, fns in tabs.items()
        }

    _bacc_mod.get_activation_tables = _only_combined
    try:
        nc.finalize()
    finally:
        _bacc_mod.get_activation_tables = real_get
    return nc


def _get_program():
    global _BUILT
    if _BUILT is None:
        _BUILT = _build_program()
    return _BUILT


def kernel(phi, idx_durations, events):
    phi = np.ascontiguousarray(np.asarray(phi), dtype=np.float32)
    d = np.asarray(idx_durations).astype(np.int64)
    e = np.asarray(events).astype(np.int64)
    u = (e > 0).astype(np.int64)
    st = np.clip(e - 1, 0, QCAUSE - 1)

    nc = _get_program()

    in_maps = []
    for c in range(N_CORES):
        sl = slice(c * S, (c + 1) * S)
        dthr = d[sl].reshape(T, 128).T.astype(np.float32)
        in_maps.append(
            {
                "phi": phi[sl].reshape(S, ROW),
                "dthr": np.ascontiguousarray(dthr),
            }
        )

    trace = os.environ.get("BASS_PROFILE") == "1"
    kw = {}
    if trace:
        tmpdir = os.environ.get("BASS_TRACE_DIR") or None
        kw = dict(trace=True, tmpdir=tmpdir)
    res = run_bass_kernel_spmd(nc, in_maps, list(range(N_CORES)), **kw)
    if trace and res.exec_time_ns is not None:
        print(f"HW exec time: {res.exec_time_ns} ns", file=sys.stderr)

    total = 0.0
    for c in range(N_CORES):
        acc = np.asarray(res.results[c]["acc_out"], dtype=np.float64)
        total += acc.sum()

    # Host tail: the two per-sample point gathers -u*(s[d] + phi[st,d])
    # and the affine constant (u - d - 1). O(N) numpy index work on data
    # the device has already streamed in full.
    phv = phi.reshape(N, QCAUSE, K)
    at_d = np.take_along_axis(phv, d[:, None, None], axis=2)[:, :, 0]  # [N, 4]
    s_at_d = at_d.sum(axis=1, dtype=np.float64)
    phi_std = at_d[np.arange(N), st].astype(np.float64)
    total -= float((u * (s_at_d + phi_std)).sum())
    total += float((u - d - 1).sum())
    return np.float32(total / N)


if __name__ == "__main__":
    rng = np.random.default_rng(0)
    phi = rng.standard_normal((N, QCAUSE, K), dtype=np.float32)
    d = rng.integers(0, K, size=(N,)).astype(np.int64)
    e = rng.integers(0, QCAUSE + 1, size=(N,)).astype(np.int64)
    print(kernel(phi, d, e))
